# revision 1
# baseline (speedup 1.0000x reference)
"""Trainium2 Bass kernel for fused embedding-lookup -> mean-pool -> dot(weights).

Reference computation (B=16384, L=200, D=100, V=100000):
    out[b] = mean_l(embed_table[word_idxs[b, l], :]) @ weights            # [B, 1]

Key algebraic transform: the dot with `weights` is linear, so
    out[b] = sum_l s[word_idxs[b, l]],   with  s = embed_table @ (weights / L)
Instead of gathering B*L rows of 400B (1.31 GB), we precompute the V-element
vector `s` on-device (the 40MB table is read exactly once across the 8 cores)
and gather B*L scalars.

The scalar gather uses the TIE-ucode `dma_gather` (int16 row indices, 256B
elements, 4 SWDGE queues). To avoid a 64-wide on-chip select per token, we
materialize a phase-shifted fat-row table in DRAM:
    S16[j, k] = s_pad[4*j + k],  j in [0, 25000), k in [0, 64)
(dense 256B rows; s_pad = s with a 32-element zero lead pad). For a token with
index v, row j = v >> 2 contains s[v] at lane 30 + (v & 3) — a fixed 4-lane
window, so the select is a 4-wide mask+reduce (~40us DVE total).

Sharding (8 cores): batch-parallel gather (2048 rows/core); vocab-parallel s
precompute (12544 padded rows/core) + AllGather.

Host does layout only: shard/reshape inputs, compute j = idx>>2 / r = idx&3,
wrap indices in the dma_gather [16, S] layout, and concat per-core outputs.
"""

import os
import sys

import numpy as np

for _p in ("/opt/trn_rl_repo",):
    if os.path.isdir(_p) and _p not in sys.path:
        sys.path.insert(0, _p)

from concourse import bacc, bass, mybir, tile  # noqa: E402
from concourse.bass_utils import run_bass_kernel_spmd  # noqa: E402

F32 = mybir.dt.float32
I32 = mybir.dt.int32
I16 = mybir.dt.int16
P = 128
NCORES = 8


def dma_gather_raw(
    gp, out_ap, in_ap, idxs_ap, num_idxs, num_idxs_reg, elem_size, elem_step,
    queue_num=0,
):
    """nc.gpsimd.dma_gather minus the 256B *element* restriction.

    Only the source row PITCH must be a 256B multiple (stride_bytes_256 is an
    8-bit field in 256B units); the per-index element payload can be smaller.
    Emits the same InstDMAGatherAnt the stock wrapper does.
    """
    dt_sz = mybir.dt.size(in_ap.dtype)
    stride_256 = (elem_step * dt_sz) // 256
    assert elem_step * dt_sz == stride_256 * 256 and 0 < stride_256 < 256
    assert in_ap.ap[0][0] == elem_step and in_ap.ap[-1][1] == elem_size
    _in_ap = gp.lower_ap_dma(in_ap, for_custom_bir_dma=True)
    _idxs_ap = gp.lower_ap(idxs_ap)
    _out_ap = gp.lower_ap(out_ap)
    return gp.add_instruction(
        mybir.InstDMAGatherAnt(
            name=gp.bass.get_next_instruction_name(),
            ins=[*_in_ap, _idxs_ap, gp.lower_val_access(gp.to_reg(num_idxs_reg))],
            outs=[_out_ap],
            transpose=False,
            num_idxs=num_idxs,
            elem_size=elem_size,
            stride_bytes_256=stride_256,
            gen_mode=0,
            single_packet=False,
            queue_num=queue_num,
            sbuf_tokens_per_rank=0,
            sbuf_free_dim_per_rank=0,
            sbuf_free_dim_pad_per_rank=0,
            sbuf_byte_offset=0,
        )
    )


def build_program(
    G=16, L=200, D=100, RPP=98, CPI=100, NQ=4, ncores=NCORES, use_collective=True,
    repeat=1, ELEM=4, GAT_BUFS=2,
):
    """Build the SPMD program (identical on all cores).

    G:   row-groups per core (batch rows per core = G*128)
    L:   tokens per row
    D:   embedding dim
    RPP: padded vocab rows per SBUF partition (vocab rows per core = 128*RPP)
    CPI: token slots (out columns) per dma_gather instruction; L % CPI == 0
    NQ:  SWDGE queues to rotate over (1..4)
    """
    assert L % CPI == 0
    SLOTS = G * L  # token slots per partition
    NT = SLOTS // CPI  # dma_gather instructions
    H = L // CPI  # instructions per row-group
    NI = P * CPI  # indices per instruction
    VPC = P * RPP
    V_PAD = VPC * ncores
    NROWS = V_PAD // 4  # row j holds s[4j .. 4j+4)
    nc = bacc.Bacc(
        "TRN2",
        target_bir_lowering=False,
        debug=False,
        num_devices=ncores,
        num_swdge_queues=NQ,
    )
    idxw_t = nc.dram_tensor("idxw", [P, SLOTS * 8], I16, kind="ExternalInput")
    io4_t = nc.dram_tensor("io4", [P, 4], F32, kind="ExternalInput")
    r2_t = nc.dram_tensor("r2", [P, SLOTS], F32, kind="ExternalInput")
    tab_t = nc.dram_tensor("tab", [P, RPP * D], F32, kind="ExternalInput")
    w_t = nc.dram_tensor("w", [P, D], F32, kind="ExternalInput")
    out_t = nc.dram_tensor("out", [P, G], F32, kind="ExternalOutput")

    with tile.TileContext(nc) as tc:
        with tc.tile_pool(name="dr", bufs=1, space="DRAM") as dr:
            with tc.tile_pool(name="pre", bufs=1) as pre:
                # ---- stage 1: s_part = (table slice) @ (w/L) ----
                tab_sb = pre.tile([P, RPP * D], F32)
                nc.sync.dma_start(tab_sb[:], tab_t[:])
                w_sb = pre.tile([P, D], F32)
                nc.sync.dma_start(w_sb[:], w_t[:])
                prod_sb = pre.tile([P, RPP * D], F32)
                nc.vector.tensor_tensor(
                    out=prod_sb[:].rearrange("p (r d) -> p r d", d=D),
                    in0=tab_sb[:].rearrange("p (r d) -> p r d", d=D),
                    in1=w_sb[:].unsqueeze(1).to_broadcast([P, RPP, D]),
                    op=mybir.AluOpType.mult,
                )
                s_sb = pre.tile([P, RPP], F32)
                nc.vector.tensor_reduce(
                    out=s_sb[:].unsqueeze(2),
                    in_=prod_sb[:].rearrange("p (r d) -> p r d", d=D),
                    axis=mybir.AxisListType.X,
                    op=mybir.AluOpType.add,
                )

                # ---- stage 2: AllGather s ----
                s_part = dr.tile([P, RPP], F32)
                nc.sync.dma_start(s_part[:], s_sb[:])
                if use_collective:
                    s_full = dr.tile([ncores * RPP, P], F32, addr_space="Shared")
                    nc.gpsimd.collective_compute(
                        "AllGather",
                        mybir.AluOpType.bypass,
                        replica_groups=[list(range(ncores))],
                        ins=[s_part.opt()],
                        outs=[s_full.opt()],
                    )
                else:
                    # crash-isolation mode: fill s_full with the local part
                    # replicated (wrong data, same program shape)
                    s_full = dr.tile([ncores * RPP, P], F32)
                    for c in range(ncores):
                        nc.sync.dma_start(
                            s_full[c * RPP : (c + 1) * RPP, :],
                            s_part[:].rearrange("p r -> (p r)").rearrange(
                                "(r q) -> r q", q=P
                            ),
                        )

                # ---- stage 3: spread s into 256B-pitch rows ----
                # S16[j, 0:4] = s[4j .. 4j+4); rows pitched 64 f32 = 256B so
                # the gather row stride is ISA-encodable; lanes 4..63 are
                # never written nor read. Token v -> row v>>2, lane v&3.
                S16 = dr.tile([NROWS, 64], F32)
                s_flat = s_full[:].rearrange("a b -> (a b)")
                # chunk: large descriptor counts in one SWDGE dma_start
                # overflow the SDMA packet machinery (HW crash above ~1k
                # descriptors per instruction).
                row = 0
                while row < NROWS:
                    n = min(1000, NROWS - row)
                    src_view = bass.AP(s_flat.tensor, 4 * row, [[4, n], [1, 4]])
                    nc.sync.dma_start(S16[row : row + n, 0:4], src_view)
                    row += n

            with (
                tc.tile_pool(name="keep", bufs=1) as keep,
                tc.tile_pool(name="gat", bufs=GAT_BUFS) as gat,
            ):
                # ---- stage 4: gather + select + reduce ----
                iota4 = keep.tile([P, 4], F32)
                nc.sync.dma_start(iota4[:], io4_t[:])
                r2_sb = keep.tile([P, SLOTS], F32)
                nc.sync.dma_start(r2_sb[:], r2_t[:])
                half_sb = keep.tile([P, NT], F32)
                out_sb = keep.tile([P, G], F32)
                iota_view = iota4[:].unsqueeze(1).to_broadcast([P, CPI, 4])
                for t in range(NT * repeat):
                    t = t % NT
                    idxw_sb = gat.tile([P, NI // 16], I16, tag="idxw", name=f"idxw{t}")
                    nc.sync.dma_start(
                        idxw_sb[:], idxw_t[:, t * (NI // 16) : (t + 1) * (NI // 16)]
                    )
                    gth = gat.tile([P, CPI, ELEM], F32, tag="gth", name=f"gth{t}")
                    dma_gather_raw(
                        nc.gpsimd,
                        gth[:],
                        S16[:, 0:ELEM],
                        idxw_sb[:],
                        NI,
                        NI,
                        elem_size=ELEM,
                        elem_step=64,
                        queue_num=t % NQ,
                    )
                    mask = gat.tile([P, CPI, 4], F32, tag="mask", name=f"mask{t}")
                    nc.vector.tensor_tensor(
                        out=mask[:],
                        in0=r2_sb[:, t * CPI : (t + 1) * CPI]
                        .unsqueeze(2)
                        .to_broadcast([P, CPI, 4]),
                        in1=iota_view,
                        op=mybir.AluOpType.is_equal,
                    )
                    msel = gat.tile([P, CPI, 4], F32, tag="msel", name=f"msel{t}")
                    nc.vector.tensor_tensor(
                        out=msel[:],
                        in0=mask[:],
                        in1=gth[:, :, 0:4],
                        op=mybir.AluOpType.mult,
                    )
                    nc.vector.tensor_reduce(
                        out=half_sb[:, t : t + 1],
                        in_=msel[:].rearrange("p a b -> p (a b)"),
                        axis=mybir.AxisListType.X,
                        op=mybir.AluOpType.add,
                    )
                nc.vector.tensor_reduce(
                    out=out_sb[:].unsqueeze(2),
                    in_=half_sb[:].rearrange("p (g h) -> p g h", h=H),
                    axis=mybir.AxisListType.X,
                    op=mybir.AluOpType.add,
                )
                nc.sync.dma_start(out_t[:], out_sb[:])
    nc.compile()
    return nc


def make_in_maps(word_idxs, embed_table, weights, G, L, D, RPP, CPI, ncores=NCORES):
    """Shard + lay out the full inputs for the per-core program."""
    BPC = G * P
    SLOTS = G * L
    NT = SLOTS // CPI
    VPC = P * RPP
    idx = np.asarray(word_idxs).astype(np.int32)
    tab = np.asarray(embed_table, dtype=np.float32)
    w = np.asarray(weights, dtype=np.float32).reshape(-1)
    V = tab.shape[0]
    tab_pad = np.zeros((VPC * ncores, D), dtype=np.float32)
    tab_pad[:V] = tab
    w_c = np.ascontiguousarray(
        np.broadcast_to((w / np.float32(L))[None, :], (P, D))
    ).astype(np.float32)
    in_maps = []
    for c in range(ncores):
        # token slot layout: [partition p, slot j=g*L+l] holds idx of batch
        # row (c*BPC + g*128 + p), token l
        slots = (
            idx[c * BPC : (c + 1) * BPC]
            .reshape(G, P, L)
            .transpose(1, 0, 2)
            .reshape(P, SLOTS)
        )
        jmat = (slots >> 2).astype(np.int16)  # [P, SLOTS]
        r2 = (slots & 3).astype(np.float32)
        # per-instruction index lists in i = c_local*128 + p order, wrapped
        # into the dma_gather [16, NI//16] layout, replicated to 128 parts
        u = jmat.reshape(P, NT, CPI).transpose(1, 2, 0)  # [NT, CPI, P]
        wrp = u.reshape(NT, CPI * P // 16, 16).transpose(2, 0, 1).reshape(16, -1)
        idxw = np.ascontiguousarray(np.tile(wrp, (8, 1)))  # [128, SLOTS*8]
        tab_c = np.ascontiguousarray(
            tab_pad[c * VPC : (c + 1) * VPC].reshape(P, RPP * D)
        )
        in_maps.append(
            {
                "idxw": idxw,
                "r2": np.ascontiguousarray(r2),
                "tab": tab_c,
                "w": w_c,
                "io4": np.ascontiguousarray(
                    np.broadcast_to(np.arange(4, dtype=np.float32), (P, 4))
                ),
            }
        )
    return in_maps


def unshard_out(results, G, ncores=NCORES):
    """results: list of per-core {'out': [128, G]} -> full [B, 1] f32."""
    parts = []
    for c in range(ncores):
        o = np.asarray(results[c]["out"])  # [P, G]; out[p, g] = row g*128+p
        parts.append(o.T.reshape(-1))
    return np.concatenate(parts).reshape(-1, 1).astype(np.float32)


_CACHED_NC = None

FULL = dict(G=16, L=200, D=100, RPP=98, CPI=100)


def _get_nc():
    global _CACHED_NC
    if _CACHED_NC is None:
        _CACHED_NC = build_program(**FULL)
    return _CACHED_NC


def run(word_idxs, embed_table, weights, trace=False, **spmd_kwargs):
    """Build (cached), run on the 8 cores, return (full_out, BassKernelResults)."""
    nc = _get_nc()
    in_maps = make_in_maps(
        word_idxs,
        embed_table,
        weights,
        FULL["G"],
        FULL["L"],
        FULL["D"],
        FULL["RPP"],
        FULL["CPI"],
    )
    res = run_bass_kernel_spmd(
        nc, in_maps, core_ids=list(range(NCORES)), trace=trace, **spmd_kwargs
    )
    out = unshard_out(res.results, FULL["G"])
    return out, res


def kernel(word_idxs, embed_table, weights):
    out, _ = run(word_idxs, embed_table, weights, trace=False)
    return out



# revision 3
# speedup vs baseline: 1.6118x; 1.6118x over previous
"""Trainium2 Bass kernel for fused embedding-lookup -> mean-pool -> dot(weights).

Reference computation (B=16384, L=200, D=100, V=100000):
    out[b] = mean_l(embed_table[word_idxs[b, l], :]) @ weights            # [B, 1]

Key algebraic transform: the dot with `weights` is linear, so
    out[b] = sum_l s[word_idxs[b, l]],   with  s = embed_table @ (weights / L)
Instead of gathering B*L rows of 400B (1.31 GB), we precompute the V-element
vector `s` on-device (the 40MB table is read exactly once across the 8 cores)
and gather B*L scalars.

The scalar gather uses the TIE-ucode `dma_gather` (int16 row indices, 256B
elements, 4 SWDGE queues). To avoid a 64-wide on-chip select per token, we
materialize a phase-shifted fat-row table in DRAM:
    S16[j, k] = s_pad[4*j + k],  j in [0, 25000), k in [0, 64)
(dense 256B rows; s_pad = s with a 32-element zero lead pad). For a token with
index v, row j = v >> 2 contains s[v] at lane 30 + (v & 3) — a fixed 4-lane
window, so the select is a 4-wide mask+reduce (~40us DVE total).

Sharding (8 cores): batch-parallel gather (2048 rows/core); vocab-parallel s
precompute (12544 padded rows/core) + AllGather.

Host does layout only: shard/reshape inputs, compute j = idx>>2 / r = idx&3,
wrap indices in the dma_gather [16, S] layout, and concat per-core outputs.
"""

import os
import sys

import numpy as np

for _p in ("/opt/trn_rl_repo",):
    if os.path.isdir(_p) and _p not in sys.path:
        sys.path.insert(0, _p)

from concourse import bacc, bass, mybir, tile  # noqa: E402
from concourse.bass_utils import run_bass_kernel_spmd  # noqa: E402

F32 = mybir.dt.float32
I32 = mybir.dt.int32
I16 = mybir.dt.int16
P = 128
NCORES = 8


def dma_gather_raw(
    gp, out_ap, in_ap, idxs_ap, num_idxs, num_idxs_reg, elem_size, elem_step,
    queue_num=0,
):
    """nc.gpsimd.dma_gather minus the 256B *element* restriction.

    Only the source row PITCH must be a 256B multiple (stride_bytes_256 is an
    8-bit field in 256B units); the per-index element payload can be smaller.
    Emits the same InstDMAGatherAnt the stock wrapper does.
    """
    dt_sz = mybir.dt.size(in_ap.dtype)
    stride_256 = (elem_step * dt_sz) // 256
    assert elem_step * dt_sz == stride_256 * 256 and 0 < stride_256 < 256
    assert in_ap.ap[0][0] == elem_step and in_ap.ap[-1][1] == elem_size
    _in_ap = gp.lower_ap_dma(in_ap, for_custom_bir_dma=True)
    _idxs_ap = gp.lower_ap(idxs_ap)
    _out_ap = gp.lower_ap(out_ap)
    return gp.add_instruction(
        mybir.InstDMAGatherAnt(
            name=gp.bass.get_next_instruction_name(),
            ins=[*_in_ap, _idxs_ap, gp.lower_val_access(gp.to_reg(num_idxs_reg))],
            outs=[_out_ap],
            transpose=False,
            num_idxs=num_idxs,
            elem_size=elem_size,
            stride_bytes_256=stride_256,
            gen_mode=0,
            single_packet=False,
            queue_num=queue_num,
            sbuf_tokens_per_rank=0,
            sbuf_free_dim_per_rank=0,
            sbuf_free_dim_pad_per_rank=0,
            sbuf_byte_offset=0,
        )
    )


def build_program(
    G=16, L=200, D=100, RPP=98, CPI=100, NQ=4, ncores=NCORES, use_collective=True,
    repeat=1, ELEM=4, GAT_BUFS=6,
):
    """Build the SPMD program (identical on all cores).

    G:   row-groups per core (batch rows per core = G*128)
    L:   tokens per row
    D:   embedding dim
    RPP: padded vocab rows per SBUF partition (vocab rows per core = 128*RPP)
    CPI: token slots (out columns) per dma_gather instruction; L % CPI == 0
    NQ:  SWDGE queues to rotate over (1..4)
    """
    assert L % CPI == 0
    SLOTS = G * L  # token slots per partition
    NT = SLOTS // CPI  # dma_gather instructions
    H = L // CPI  # instructions per row-group
    NI = P * CPI  # indices per instruction
    VPC = P * RPP
    V_PAD = VPC * ncores
    NROWS = V_PAD // 4  # row j holds s[4j .. 4j+4)
    nc = bacc.Bacc(
        "TRN2",
        target_bir_lowering=False,
        debug=False,
        num_devices=ncores,
        num_swdge_queues=NQ,
    )
    idxw_t = nc.dram_tensor("idxw", [P, SLOTS * 8], I16, kind="ExternalInput")
    io4_t = nc.dram_tensor("io4", [P, 4], F32, kind="ExternalInput")
    r2_t = nc.dram_tensor("r2", [P, SLOTS], F32, kind="ExternalInput")
    tab_t = nc.dram_tensor("tab", [P, RPP * D], F32, kind="ExternalInput")
    w_t = nc.dram_tensor("w", [P, D], F32, kind="ExternalInput")
    out_t = nc.dram_tensor("out", [P, G], F32, kind="ExternalOutput")

    with tile.TileContext(nc) as tc:
        with tc.tile_pool(name="dr", bufs=1, space="DRAM") as dr:
            with tc.tile_pool(name="pre", bufs=1) as pre:
                # ---- stage 1: s_part = (table slice) @ (w/L) ----
                tab_sb = pre.tile([P, RPP * D], F32)
                nc.sync.dma_start(tab_sb[:], tab_t[:])
                w_sb = pre.tile([P, D], F32)
                nc.sync.dma_start(w_sb[:], w_t[:])
                prod_sb = pre.tile([P, RPP * D], F32)
                nc.vector.tensor_tensor(
                    out=prod_sb[:].rearrange("p (r d) -> p r d", d=D),
                    in0=tab_sb[:].rearrange("p (r d) -> p r d", d=D),
                    in1=w_sb[:].unsqueeze(1).to_broadcast([P, RPP, D]),
                    op=mybir.AluOpType.mult,
                )
                s_sb = pre.tile([P, RPP], F32)
                nc.vector.tensor_reduce(
                    out=s_sb[:].unsqueeze(2),
                    in_=prod_sb[:].rearrange("p (r d) -> p r d", d=D),
                    axis=mybir.AxisListType.X,
                    op=mybir.AluOpType.add,
                )

                # ---- stage 2: AllGather s ----
                s_part = dr.tile([P, RPP], F32)
                nc.sync.dma_start(s_part[:], s_sb[:])
                if use_collective:
                    s_full = dr.tile([ncores * RPP, P], F32, addr_space="Shared")
                    nc.gpsimd.collective_compute(
                        "AllGather",
                        mybir.AluOpType.bypass,
                        replica_groups=[list(range(ncores))],
                        ins=[s_part.opt()],
                        outs=[s_full.opt()],
                    )
                else:
                    # crash-isolation mode: fill s_full with the local part
                    # replicated (wrong data, same program shape)
                    s_full = dr.tile([ncores * RPP, P], F32)
                    for c in range(ncores):
                        nc.sync.dma_start(
                            s_full[c * RPP : (c + 1) * RPP, :],
                            s_part[:].rearrange("p r -> (p r)").rearrange(
                                "(r q) -> r q", q=P
                            ),
                        )

                # ---- stage 3: spread s into 256B-pitch rows ----
                # S16[j, 0:4] = s[4j .. 4j+4); rows pitched 64 f32 = 256B so
                # the gather row stride is ISA-encodable; lanes 4..63 are
                # never written nor read. Token v -> row v>>2, lane v&3.
                #
                # Build the spread layout in SBUF with one strided DVE copy,
                # then ship it to DRAM as a single dense 6.4MB transfer. The
                # per-row descriptor route (25088 16B descriptors) costs ~69us
                # of HWDGE/SDMA time; this costs ~20us.
                RPP64 = NROWS // P  # spread rows per partition
                S16 = dr.tile([NROWS, 64], F32)
                sfull_sb = pre.tile([P, RPP64 * 4], F32)
                nc.sync.dma_start(
                    sfull_sb[:],
                    s_full[:]
                    .rearrange("a b -> (a b)")
                    .rearrange("(p x) -> p x", p=P),
                )
                ssp_sb = pre.tile([P, RPP64 * 64], F32)
                # zero-fill so the dense DRAM write doesn't ship uninitialized
                # SBUF (lanes 4..63 are never read back, but the tile checker
                # wants them defined).
                nc.vector.memset(ssp_sb[:], 0.0)
                nc.vector.tensor_copy(
                    out=ssp_sb[:].rearrange("p (r k) -> p r k", k=64)[:, :, 0:4],
                    in_=sfull_sb[:].rearrange("p (r q) -> p r q", q=4),
                )
                nc.sync.dma_start(
                    S16[:].rearrange("a b -> (a b)").rearrange("(p x) -> p x", p=P),
                    ssp_sb[:],
                )

            with (
                tc.tile_pool(name="keep", bufs=1) as keep,
                tc.tile_pool(name="gat", bufs=GAT_BUFS) as gat,
            ):
                # ---- stage 4: gather + select + reduce ----
                iota4 = keep.tile([P, 4], F32)
                nc.sync.dma_start(iota4[:], io4_t[:])
                r2_sb = keep.tile([P, SLOTS], F32)
                nc.sync.dma_start(r2_sb[:], r2_t[:])
                half_sb = keep.tile([P, NT], F32)
                out_sb = keep.tile([P, G], F32)
                iota_view = iota4[:].unsqueeze(1).to_broadcast([P, CPI, 4])
                for t in range(NT * repeat):
                    t = t % NT
                    idxw_sb = gat.tile([P, NI // 16], I16, tag="idxw", name=f"idxw{t}")
                    nc.sync.dma_start(
                        idxw_sb[:], idxw_t[:, t * (NI // 16) : (t + 1) * (NI // 16)]
                    )
                    gth = gat.tile([P, CPI, ELEM], F32, tag="gth", name=f"gth{t}")
                    dma_gather_raw(
                        nc.gpsimd,
                        gth[:],
                        S16[:, 0:ELEM],
                        idxw_sb[:],
                        NI,
                        NI,
                        elem_size=ELEM,
                        elem_step=64,
                        queue_num=t % NQ,
                    )
                    mask = gat.tile([P, CPI, 4], F32, tag="mask", name=f"mask{t}")
                    nc.vector.tensor_tensor(
                        out=mask[:],
                        in0=r2_sb[:, t * CPI : (t + 1) * CPI]
                        .unsqueeze(2)
                        .to_broadcast([P, CPI, 4]),
                        in1=iota_view,
                        op=mybir.AluOpType.is_equal,
                    )
                    msel = gat.tile([P, CPI, 4], F32, tag="msel", name=f"msel{t}")
                    nc.vector.tensor_tensor(
                        out=msel[:],
                        in0=mask[:],
                        in1=gth[:, :, 0:4],
                        op=mybir.AluOpType.mult,
                    )
                    nc.vector.tensor_reduce(
                        out=half_sb[:, t : t + 1],
                        in_=msel[:].rearrange("p a b -> p (a b)"),
                        axis=mybir.AxisListType.X,
                        op=mybir.AluOpType.add,
                    )
                nc.vector.tensor_reduce(
                    out=out_sb[:].unsqueeze(2),
                    in_=half_sb[:].rearrange("p (g h) -> p g h", h=H),
                    axis=mybir.AxisListType.X,
                    op=mybir.AluOpType.add,
                )
                nc.sync.dma_start(out_t[:], out_sb[:])
    nc.compile()
    return nc


def make_in_maps(word_idxs, embed_table, weights, G, L, D, RPP, CPI, ncores=NCORES):
    """Shard + lay out the full inputs for the per-core program."""
    BPC = G * P
    SLOTS = G * L
    NT = SLOTS // CPI
    VPC = P * RPP
    idx = np.asarray(word_idxs).astype(np.int32)
    tab = np.asarray(embed_table, dtype=np.float32)
    w = np.asarray(weights, dtype=np.float32).reshape(-1)
    V = tab.shape[0]
    tab_pad = np.zeros((VPC * ncores, D), dtype=np.float32)
    tab_pad[:V] = tab
    w_c = np.ascontiguousarray(
        np.broadcast_to((w / np.float32(L))[None, :], (P, D))
    ).astype(np.float32)
    in_maps = []
    for c in range(ncores):
        # token slot layout: [partition p, slot j=g*L+l] holds idx of batch
        # row (c*BPC + g*128 + p), token l
        slots = (
            idx[c * BPC : (c + 1) * BPC]
            .reshape(G, P, L)
            .transpose(1, 0, 2)
            .reshape(P, SLOTS)
        )
        jmat = (slots >> 2).astype(np.int16)  # [P, SLOTS]
        r2 = (slots & 3).astype(np.float32)
        # per-instruction index lists in i = c_local*128 + p order, wrapped
        # into the dma_gather [16, NI//16] layout, replicated to 128 parts
        u = jmat.reshape(P, NT, CPI).transpose(1, 2, 0)  # [NT, CPI, P]
        wrp = u.reshape(NT, CPI * P // 16, 16).transpose(2, 0, 1).reshape(16, -1)
        idxw = np.ascontiguousarray(np.tile(wrp, (8, 1)))  # [128, SLOTS*8]
        tab_c = np.ascontiguousarray(
            tab_pad[c * VPC : (c + 1) * VPC].reshape(P, RPP * D)
        )
        in_maps.append(
            {
                "idxw": idxw,
                "r2": np.ascontiguousarray(r2),
                "tab": tab_c,
                "w": w_c,
                "io4": np.ascontiguousarray(
                    np.broadcast_to(np.arange(4, dtype=np.float32), (P, 4))
                ),
            }
        )
    return in_maps


def unshard_out(results, G, ncores=NCORES):
    """results: list of per-core {'out': [128, G]} -> full [B, 1] f32."""
    parts = []
    for c in range(ncores):
        o = np.asarray(results[c]["out"])  # [P, G]; out[p, g] = row g*128+p
        parts.append(o.T.reshape(-1))
    return np.concatenate(parts).reshape(-1, 1).astype(np.float32)


_CACHED_NC = None

FULL = dict(G=16, L=200, D=100, RPP=98, CPI=100)


def _get_nc():
    global _CACHED_NC
    if _CACHED_NC is None:
        _CACHED_NC = build_program(**FULL)
    return _CACHED_NC


def run(word_idxs, embed_table, weights, trace=False, **spmd_kwargs):
    """Build (cached), run on the 8 cores, return (full_out, BassKernelResults)."""
    nc = _get_nc()
    in_maps = make_in_maps(
        word_idxs,
        embed_table,
        weights,
        FULL["G"],
        FULL["L"],
        FULL["D"],
        FULL["RPP"],
        FULL["CPI"],
    )
    res = run_bass_kernel_spmd(
        nc, in_maps, core_ids=list(range(NCORES)), trace=trace, **spmd_kwargs
    )
    out = unshard_out(res.results, FULL["G"])
    return out, res


def kernel(word_idxs, embed_table, weights):
    out, _ = run(word_idxs, embed_table, weights, trace=False)
    return out



# revision 9
# speedup vs baseline: 1.7597x; 1.0917x over previous
"""Trainium2 Bass kernel for fused embedding-lookup -> mean-pool -> dot(weights).

Reference computation (B=16384, L=200, D=100, V=100000):
    out[b] = mean_l(embed_table[word_idxs[b, l], :]) @ weights            # [B, 1]

Key algebraic transform: the dot with `weights` is linear, so
    out[b] = sum_l s[word_idxs[b, l]],   with  s = embed_table @ (weights / L)
Instead of gathering B*L rows of 400B (1.31 GB), we precompute the V-element
vector `s` on-device (the 40MB table is read exactly once across the 8 cores)
and gather B*L scalars.

The scalar gather uses the TIE-ucode `dma_gather` (int16 row indices, 256B
elements, 4 SWDGE queues). To avoid a 64-wide on-chip select per token, we
materialize a phase-shifted fat-row table in DRAM:
    S16[j, k] = s_pad[4*j + k],  j in [0, 25000), k in [0, 64)
(dense 256B rows; s_pad = s with a 32-element zero lead pad). For a token with
index v, row j = v >> 2 contains s[v] at lane 30 + (v & 3) — a fixed 4-lane
window, so the select is a 4-wide mask+reduce (~40us DVE total).

Sharding (8 cores): batch-parallel gather (2048 rows/core); vocab-parallel s
precompute (12544 padded rows/core) + AllGather.

Host does layout only: shard/reshape inputs, compute j = idx>>2 / r = idx&3,
wrap indices in the dma_gather [16, S] layout, and concat per-core outputs.
"""

import os
import sys

import numpy as np

for _p in ("/opt/trn_rl_repo",):
    if os.path.isdir(_p) and _p not in sys.path:
        sys.path.insert(0, _p)

from concourse import bacc, bass, mybir, tile  # noqa: E402
from concourse.bass_utils import run_bass_kernel_spmd  # noqa: E402

F32 = mybir.dt.float32
BF16 = mybir.dt.bfloat16
I32 = mybir.dt.int32
I16 = mybir.dt.int16
P = 128
NCORES = 8


def dma_gather_raw(
    gp, out_ap, in_ap, idxs_ap, num_idxs, num_idxs_reg, elem_size, elem_step,
    queue_num=0,
):
    """nc.gpsimd.dma_gather minus the 256B *element* restriction.

    Only the source row PITCH must be a 256B multiple (stride_bytes_256 is an
    8-bit field in 256B units); the per-index element payload can be smaller.
    Emits the same InstDMAGatherAnt the stock wrapper does.
    """
    dt_sz = mybir.dt.size(in_ap.dtype)
    stride_256 = (elem_step * dt_sz) // 256
    assert elem_step * dt_sz == stride_256 * 256 and 0 < stride_256 < 256
    assert in_ap.ap[0][0] == elem_step and in_ap.ap[-1][1] == elem_size
    _in_ap = gp.lower_ap_dma(in_ap, for_custom_bir_dma=True)
    _idxs_ap = gp.lower_ap(idxs_ap)
    _out_ap = gp.lower_ap(out_ap)
    return gp.add_instruction(
        mybir.InstDMAGatherAnt(
            name=gp.bass.get_next_instruction_name(),
            ins=[*_in_ap, _idxs_ap, gp.lower_val_access(gp.to_reg(num_idxs_reg))],
            outs=[_out_ap],
            transpose=False,
            num_idxs=num_idxs,
            elem_size=elem_size,
            stride_bytes_256=stride_256,
            gen_mode=0,
            single_packet=False,
            queue_num=queue_num,
            sbuf_tokens_per_rank=0,
            sbuf_free_dim_per_rank=0,
            sbuf_free_dim_pad_per_rank=0,
            sbuf_byte_offset=0,
        )
    )


def build_program(
    G=16, L=200, D=100, RPP=98, CPI=100, NQ=4, ncores=NCORES, use_collective=True,
    repeat=1, ELEM=4, GAT_BUFS=8,
):
    """Build the SPMD program (identical on all cores).

    G:   row-groups per core (batch rows per core = G*128)
    L:   tokens per row
    D:   embedding dim
    RPP: padded vocab rows per SBUF partition (vocab rows per core = 128*RPP)
    CPI: token slots (out columns) per dma_gather instruction; L % CPI == 0
    NQ:  SWDGE queues to rotate over (1..4)
    """
    assert L % CPI == 0
    SLOTS = G * L  # token slots per partition
    NT = SLOTS // CPI  # dma_gather instructions
    H = L // CPI  # instructions per row-group
    NI = P * CPI  # indices per instruction
    VPC = P * RPP
    V_PAD = VPC * ncores
    NROWS = V_PAD // 4  # row j holds s[4j .. 4j+4)
    nc = bacc.Bacc(
        "TRN2",
        target_bir_lowering=False,
        debug=False,
        num_devices=ncores,
        num_swdge_queues=NQ,
    )
    idxw_t = nc.dram_tensor("idxw", [P, SLOTS * 8], I16, kind="ExternalInput")
    io4_t = nc.dram_tensor("io4", [P, 4], F32, kind="ExternalInput")
    r2_t = nc.dram_tensor("r2", [P, SLOTS], F32, kind="ExternalInput")
    tab_t = nc.dram_tensor("tab", [P, RPP * D], BF16, kind="ExternalInput")
    w_t = nc.dram_tensor("w", [P, D], BF16, kind="ExternalInput")
    out_t = nc.dram_tensor("out", [P, G], F32, kind="ExternalOutput")

    with tile.TileContext(nc) as tc:
        with tc.tile_pool(name="dr", bufs=1, space="DRAM") as dr:
            with tc.tile_pool(name="pre", bufs=1) as pre:
                # ---- stage 1: s_part = (table slice) @ (w/L) ----
                # bf16 table/weights: halves the 5MB load and doubles DVE
                # throughput for the product; the reduce accumulates to f32.
                tab_sb = pre.tile([P, RPP * D], BF16)
                nc.sync.dma_start(tab_sb[:], tab_t[:])
                w_sb = pre.tile([P, D], BF16)
                nc.sync.dma_start(w_sb[:], w_t[:])
                prod_sb = pre.tile([P, RPP * D], BF16)
                nc.vector.tensor_tensor(
                    out=prod_sb[:].rearrange("p (r d) -> p r d", d=D),
                    in0=tab_sb[:].rearrange("p (r d) -> p r d", d=D),
                    in1=w_sb[:].unsqueeze(1).to_broadcast([P, RPP, D]),
                    op=mybir.AluOpType.mult,
                )
                s_sb = pre.tile([P, RPP], F32)
                nc.vector.tensor_reduce(
                    out=s_sb[:].unsqueeze(2),
                    in_=prod_sb[:].rearrange("p (r d) -> p r d", d=D),
                    axis=mybir.AxisListType.X,
                    op=mybir.AluOpType.add,
                )

                # ---- stage 2: AllGather s ----
                s_part = dr.tile([P, RPP], F32)
                nc.sync.dma_start(s_part[:], s_sb[:])
                if use_collective:
                    s_full = dr.tile([ncores * RPP, P], F32, addr_space="Shared")
                    nc.gpsimd.collective_compute(
                        "AllGather",
                        mybir.AluOpType.bypass,
                        replica_groups=[list(range(ncores))],
                        ins=[s_part.opt()],
                        outs=[s_full.opt()],
                    )
                else:
                    # crash-isolation mode: fill s_full with the local part
                    # replicated (wrong data, same program shape)
                    s_full = dr.tile([ncores * RPP, P], F32)
                    for c in range(ncores):
                        nc.sync.dma_start(
                            s_full[c * RPP : (c + 1) * RPP, :],
                            s_part[:].rearrange("p r -> (p r)").rearrange(
                                "(r q) -> r q", q=P
                            ),
                        )

                # ---- stage 3: spread s into 256B-pitch rows ----
                # S16[j, 0:4] = s[4j .. 4j+4); rows pitched 64 f32 = 256B so
                # the gather row stride is ISA-encodable; lanes 4..63 are
                # never written nor read. Token v -> row v>>2, lane v&3.
                #
                # Build the spread layout in SBUF with one strided DVE copy,
                # then ship it to DRAM as a single dense 6.4MB transfer. The
                # per-row descriptor route (25088 16B descriptors) costs ~69us
                # of HWDGE/SDMA time; this costs ~20us.
                RPP64 = NROWS // P  # spread rows per partition
                S16 = dr.tile([NROWS, 64], F32)
                sfull_sb = pre.tile([P, RPP64 * 4], F32)
                nc.sync.dma_start(
                    sfull_sb[:],
                    s_full[:]
                    .rearrange("a b -> (a b)")
                    .rearrange("(p x) -> p x", p=P),
                )
                ssp_sb = pre.tile([P, RPP64 * 64], F32)
                # lanes 4..63 of each 256B row are never read by the gather
                # (in_ap covers lanes 0:ELEM only), so they ship to DRAM
                # uninitialized — no memset needed.
                nc.vector.tensor_copy(
                    out=ssp_sb[:].rearrange("p (r k) -> p r k", k=64)[:, :, 0:4],
                    in_=sfull_sb[:].rearrange("p (r q) -> p r q", q=4),
                )
                nc.sync.dma_start(
                    S16[:].rearrange("a b -> (a b)").rearrange("(p x) -> p x", p=P),
                    ssp_sb[:],
                )

            with (
                tc.tile_pool(name="keep", bufs=1) as keep,
                tc.tile_pool(name="gat", bufs=GAT_BUFS) as gat,
            ):
                # ---- stage 4: gather + select + reduce ----
                iota4 = keep.tile([P, 4], F32)
                nc.sync.dma_start(iota4[:], io4_t[:])
                r2_sb = keep.tile([P, SLOTS], F32)
                nc.sync.dma_start(r2_sb[:], r2_t[:])
                half_sb = keep.tile([P, NT], F32)
                out_sb = keep.tile([P, G], F32)
                iota_view = iota4[:].unsqueeze(1).to_broadcast([P, CPI, 4])
                for t in range(NT * repeat):
                    t = t % NT
                    idxw_sb = gat.tile([P, NI // 16], I16, tag="idxw", name=f"idxw{t}")
                    nc.sync.dma_start(
                        idxw_sb[:], idxw_t[:, t * (NI // 16) : (t + 1) * (NI // 16)]
                    )
                    gth = gat.tile([P, CPI, ELEM], F32, tag="gth", name=f"gth{t}")
                    dma_gather_raw(
                        nc.gpsimd,
                        gth[:],
                        S16[:, 0:ELEM],
                        idxw_sb[:],
                        NI,
                        NI,
                        elem_size=ELEM,
                        elem_step=64,
                        queue_num=t % NQ,
                    )
                    mask = gat.tile([P, CPI, 4], F32, tag="mask", name=f"mask{t}")
                    nc.vector.tensor_tensor(
                        out=mask[:],
                        in0=r2_sb[:, t * CPI : (t + 1) * CPI]
                        .unsqueeze(2)
                        .to_broadcast([P, CPI, 4]),
                        in1=iota_view,
                        op=mybir.AluOpType.is_equal,
                    )
                    msel = gat.tile([P, CPI, 4], F32, tag="msel", name=f"msel{t}")
                    nc.vector.tensor_tensor(
                        out=msel[:],
                        in0=mask[:],
                        in1=gth[:, :, 0:4],
                        op=mybir.AluOpType.mult,
                    )
                    nc.vector.tensor_reduce(
                        out=half_sb[:, t : t + 1],
                        in_=msel[:].rearrange("p a b -> p (a b)"),
                        axis=mybir.AxisListType.X,
                        op=mybir.AluOpType.add,
                    )
                nc.vector.tensor_reduce(
                    out=out_sb[:].unsqueeze(2),
                    in_=half_sb[:].rearrange("p (g h) -> p g h", h=H),
                    axis=mybir.AxisListType.X,
                    op=mybir.AluOpType.add,
                )
                nc.sync.dma_start(out_t[:], out_sb[:])
    nc.compile()
    return nc


def make_in_maps(word_idxs, embed_table, weights, G, L, D, RPP, CPI, ncores=NCORES):
    """Shard + lay out the full inputs for the per-core program."""
    BPC = G * P
    SLOTS = G * L
    NT = SLOTS // CPI
    VPC = P * RPP
    import ml_dtypes

    bf16 = ml_dtypes.bfloat16
    idx = np.asarray(word_idxs).astype(np.int32)
    tab = np.asarray(embed_table, dtype=np.float32)
    w = np.asarray(weights, dtype=np.float32).reshape(-1)
    V = tab.shape[0]
    tab_pad = np.zeros((VPC * ncores, D), dtype=bf16)
    tab_pad[:V] = tab.astype(bf16)
    w_c = np.ascontiguousarray(
        np.broadcast_to((w / np.float32(L))[None, :].astype(bf16), (P, D))
    )
    in_maps = []
    for c in range(ncores):
        # token slot layout: [partition p, slot j=g*L+l] holds idx of batch
        # row (c*BPC + g*128 + p), token l
        slots = (
            idx[c * BPC : (c + 1) * BPC]
            .reshape(G, P, L)
            .transpose(1, 0, 2)
            .reshape(P, SLOTS)
        )
        jmat = (slots >> 2).astype(np.int16)  # [P, SLOTS]
        r2 = (slots & 3).astype(np.float32)
        # per-instruction index lists in i = c_local*128 + p order, wrapped
        # into the dma_gather [16, NI//16] layout, replicated to 128 parts
        u = jmat.reshape(P, NT, CPI).transpose(1, 2, 0)  # [NT, CPI, P]
        wrp = u.reshape(NT, CPI * P // 16, 16).transpose(2, 0, 1).reshape(16, -1)
        idxw = np.ascontiguousarray(np.tile(wrp, (8, 1)))  # [128, SLOTS*8]
        tab_c = np.ascontiguousarray(
            tab_pad[c * VPC : (c + 1) * VPC].reshape(P, RPP * D)
        )
        in_maps.append(
            {
                "idxw": idxw,
                "r2": np.ascontiguousarray(r2),
                "tab": tab_c,
                "w": w_c,
                "io4": np.ascontiguousarray(
                    np.broadcast_to(np.arange(4, dtype=np.float32), (P, 4))
                ),
            }
        )
    return in_maps


def unshard_out(results, G, ncores=NCORES):
    """results: list of per-core {'out': [128, G]} -> full [B, 1] f32."""
    parts = []
    for c in range(ncores):
        o = np.asarray(results[c]["out"])  # [P, G]; out[p, g] = row g*128+p
        parts.append(o.T.reshape(-1))
    return np.concatenate(parts).reshape(-1, 1).astype(np.float32)


_CACHED_NC = None

FULL = dict(G=16, L=200, D=100, RPP=98, CPI=100)


def _get_nc():
    global _CACHED_NC
    if _CACHED_NC is None:
        _CACHED_NC = build_program(**FULL)
    return _CACHED_NC


def run(word_idxs, embed_table, weights, trace=False, **spmd_kwargs):
    """Build (cached), run on the 8 cores, return (full_out, BassKernelResults)."""
    nc = _get_nc()
    in_maps = make_in_maps(
        word_idxs,
        embed_table,
        weights,
        FULL["G"],
        FULL["L"],
        FULL["D"],
        FULL["RPP"],
        FULL["CPI"],
    )
    res = run_bass_kernel_spmd(
        nc, in_maps, core_ids=list(range(NCORES)), trace=trace, **spmd_kwargs
    )
    out = unshard_out(res.results, FULL["G"])
    return out, res


def kernel(word_idxs, embed_table, weights):
    out, _ = run(word_idxs, embed_table, weights, trace=False)
    return out



# revision 16
# speedup vs baseline: 1.9717x; 1.1205x over previous
"""Trainium2 Bass kernel for fused embedding-lookup -> mean-pool -> dot(weights).

Reference computation (B=16384, L=200, D=100, V=100000):
    out[b] = mean_l(embed_table[word_idxs[b, l], :]) @ weights            # [B, 1]

Key algebraic transform: the dot with `weights` is linear, so
    out[b] = sum_l s[word_idxs[b, l]],   with  s = embed_table @ (weights / L)
Instead of gathering B*L rows of 400B (1.31 GB), we precompute the V-element
vector `s` on-device (the 40MB table is read exactly once across the 8 cores)
and gather B*L scalars.

The scalar gather uses the TIE-ucode `dma_gather` (int16 row indices, 256B
elements, 4 SWDGE queues). To avoid a 64-wide on-chip select per token, we
materialize a phase-shifted fat-row table in DRAM:
    S16[j, k] = s_pad[4*j + k],  j in [0, 25000), k in [0, 64)
(dense 256B rows; s_pad = s with a 32-element zero lead pad). For a token with
index v, row j = v >> 2 contains s[v] at lane 30 + (v & 3) — a fixed 4-lane
window, so the select is a 4-wide mask+reduce (~40us DVE total).

Sharding (8 cores): batch-parallel gather (2048 rows/core); vocab-parallel s
precompute (12544 padded rows/core) + AllGather.

Host does layout only: shard/reshape inputs, compute j = idx>>2 / r = idx&3,
wrap indices in the dma_gather [16, S] layout, and concat per-core outputs.
"""

import os
import sys

import numpy as np

for _p in ("/opt/trn_rl_repo",):
    if os.path.isdir(_p) and _p not in sys.path:
        sys.path.insert(0, _p)

from concourse import bacc, bass, mybir, tile  # noqa: E402
from concourse.bass_utils import run_bass_kernel_spmd  # noqa: E402

F32 = mybir.dt.float32
BF16 = mybir.dt.bfloat16
I32 = mybir.dt.int32
I16 = mybir.dt.int16
P = 128
NCORES = 8


def dma_gather_raw(
    gp, out_ap, in_ap, idxs_ap, num_idxs, num_idxs_reg, elem_size, elem_step,
    queue_num=0, single_packet=False,
):
    """nc.gpsimd.dma_gather minus the 256B *element* restriction.

    Only the source row PITCH must be a 256B multiple (stride_bytes_256 is an
    8-bit field in 256B units); the per-index element payload can be smaller.
    Emits the same InstDMAGatherAnt the stock wrapper does.
    """
    dt_sz = mybir.dt.size(in_ap.dtype)
    stride_256 = (elem_step * dt_sz) // 256
    assert elem_step * dt_sz == stride_256 * 256 and 0 < stride_256 < 256
    assert in_ap.ap[0][0] == elem_step and in_ap.ap[-1][1] == elem_size
    _in_ap = gp.lower_ap_dma(in_ap, for_custom_bir_dma=True)
    _idxs_ap = gp.lower_ap(idxs_ap)
    _out_ap = gp.lower_ap(out_ap)
    return gp.add_instruction(
        mybir.InstDMAGatherAnt(
            name=gp.bass.get_next_instruction_name(),
            ins=[*_in_ap, _idxs_ap, gp.lower_val_access(gp.to_reg(num_idxs_reg))],
            outs=[_out_ap],
            transpose=False,
            num_idxs=num_idxs,
            elem_size=elem_size,
            stride_bytes_256=stride_256,
            gen_mode=0,
            single_packet=single_packet,
            queue_num=queue_num,
            sbuf_tokens_per_rank=0,
            sbuf_free_dim_per_rank=0,
            sbuf_free_dim_pad_per_rank=0,
            sbuf_byte_offset=0,
        )
    )


def build_program(
    G=16, L=200, D=100, RPP=98, CPI=100, NQ=4, ncores=NCORES, use_collective=True,
    repeat=1, ELEM=4, GAT_BUFS=8, SINGLE_PACKET=False,
):
    """Build the SPMD program (identical on all cores).

    G:   row-groups per core (batch rows per core = G*128)
    L:   tokens per row
    D:   embedding dim
    RPP: padded vocab rows per SBUF partition (vocab rows per core = 128*RPP)
    CPI: token-slot BLOCK size (gather instructions cover 1..4 blocks);
         L % CPI == 0
    NQ:  SWDGE queues to rotate over (1..4)

    Gather instructions are grouped into per-queue chains whose first (and
    last) instructions have staggered sizes (1/2/3/4 blocks). Same-queue
    instructions serialize through descriptor-ring space, so equal-sized
    chains would fall into lockstep: all four queues' SDMA drains collide
    after each descgen wave and every queue idles through the combined
    drain. Staggered chain heads phase-shift the queues so each queue's
    drain overlaps the other queues' descgen.
    """
    assert L % CPI == 0
    BLK = CPI  # token slots per block
    SLOTS = G * L  # token slots per partition
    NBLK = SLOTS // BLK  # total blocks
    HB = L // BLK  # blocks per row-group
    ICOL = P * BLK // 16  # idxw columns per block
    MAXB = 4  # max blocks per gather instruction
    # per-queue chains in blocks: [q+1, 4, 4, ..., remainder]
    base = NBLK // NQ
    plan = []  # (queue, first block, nblocks), in issue order
    chains = []
    for q in range(NQ):
        first = min(q + 1, base)
        rest = base - first
        sizes = [first] + [MAXB] * (rest // MAXB)
        if rest % MAXB:
            sizes.append(rest % MAXB)
        chains.append(sizes)
    blk0 = 0
    pos = [0] * NQ
    while any(pos[q] < len(chains[q]) for q in range(NQ)):
        for q in range(NQ):
            if pos[q] < len(chains[q]):
                n = chains[q][pos[q]]
                pos[q] += 1
                plan.append((q, blk0, n))
                blk0 += n
    assert blk0 == NBLK
    VPC = P * RPP
    V_PAD = VPC * ncores
    NROWS = V_PAD // 4  # row j holds s[4j .. 4j+4)
    nc = bacc.Bacc(
        "TRN2",
        target_bir_lowering=False,
        debug=False,
        num_devices=ncores,
        num_swdge_queues=NQ,
    )
    idxw_t = nc.dram_tensor("idxw", [P, SLOTS * 8], I16, kind="ExternalInput")
    io4_t = nc.dram_tensor("io4", [P, 4], F32, kind="ExternalInput")
    r2_t = nc.dram_tensor("r2", [P, SLOTS], F32, kind="ExternalInput")
    tab_t = nc.dram_tensor("tab", [P, RPP * D], BF16, kind="ExternalInput")
    w_t = nc.dram_tensor("w", [P, D], BF16, kind="ExternalInput")
    out_t = nc.dram_tensor("out", [P, G], F32, kind="ExternalOutput")

    with tile.TileContext(nc) as tc:
        with tc.tile_pool(name="dr", bufs=1, space="DRAM") as dr:
            with tc.tile_pool(name="pre", bufs=1) as pre:
                # ---- stage 1: s_part = (table slice) @ (w/L) ----
                # bf16 table/weights: halves the 5MB load and doubles DVE
                # throughput for the product; the reduce accumulates to f32.
                tab_sb = pre.tile([P, RPP * D], BF16)
                nc.sync.dma_start(tab_sb[:], tab_t[:])
                w_sb = pre.tile([P, D], BF16)
                nc.sync.dma_start(w_sb[:], w_t[:])
                prod_sb = pre.tile([P, RPP * D], BF16)
                nc.vector.tensor_tensor(
                    out=prod_sb[:].rearrange("p (r d) -> p r d", d=D),
                    in0=tab_sb[:].rearrange("p (r d) -> p r d", d=D),
                    in1=w_sb[:].unsqueeze(1).to_broadcast([P, RPP, D]),
                    op=mybir.AluOpType.mult,
                )
                s_sb = pre.tile([P, RPP], F32)
                nc.vector.tensor_reduce(
                    out=s_sb[:].unsqueeze(2),
                    in_=prod_sb[:].rearrange("p (r d) -> p r d", d=D),
                    axis=mybir.AxisListType.X,
                    op=mybir.AluOpType.add,
                )

                # ---- stage 2: AllGather s ----
                s_part = dr.tile([P, RPP], F32)
                nc.sync.dma_start(s_part[:], s_sb[:])
                if use_collective:
                    s_full = dr.tile([ncores * RPP, P], F32, addr_space="Shared")
                    nc.gpsimd.collective_compute(
                        "AllGather",
                        mybir.AluOpType.bypass,
                        replica_groups=[list(range(ncores))],
                        ins=[s_part.opt()],
                        outs=[s_full.opt()],
                    )
                else:
                    # crash-isolation mode: fill s_full with the local part
                    # replicated (wrong data, same program shape)
                    s_full = dr.tile([ncores * RPP, P], F32)
                    for c in range(ncores):
                        nc.sync.dma_start(
                            s_full[c * RPP : (c + 1) * RPP, :],
                            s_part[:].rearrange("p r -> (p r)").rearrange(
                                "(r q) -> r q", q=P
                            ),
                        )

                # ---- stage 3: spread s into 256B-pitch rows ----
                # S16[j, 0:4] = s[4j .. 4j+4); rows pitched 64 f32 = 256B so
                # the gather row stride is ISA-encodable; lanes 4..63 are
                # never written nor read. Token v -> row v>>2, lane v&3.
                #
                # Build the spread layout in SBUF with one strided DVE copy,
                # then ship it to DRAM as a single dense 6.4MB transfer. The
                # per-row descriptor route (25088 16B descriptors) costs ~69us
                # of HWDGE/SDMA time; this costs ~20us.
                RPP64 = NROWS // P  # spread rows per partition
                S16 = dr.tile([NROWS, 64], F32)
                sfull_sb = pre.tile([P, RPP64 * 4], F32)
                nc.sync.dma_start(
                    sfull_sb[:],
                    s_full[:]
                    .rearrange("a b -> (a b)")
                    .rearrange("(p x) -> p x", p=P),
                )
                ssp_sb = pre.tile([P, RPP64 * 64], F32)
                # lanes 4..63 of each 256B row are never read by the gather
                # (in_ap covers lanes 0:ELEM only), so they ship to DRAM
                # uninitialized — no memset needed.
                nc.vector.tensor_copy(
                    out=ssp_sb[:].rearrange("p (r k) -> p r k", k=64)[:, :, 0:4],
                    in_=sfull_sb[:].rearrange("p (r q) -> p r q", q=4),
                )
                nc.sync.dma_start(
                    S16[:].rearrange("a b -> (a b)").rearrange("(p x) -> p x", p=P),
                    ssp_sb[:],
                )

            with (
                tc.tile_pool(name="keep", bufs=1) as keep,
                tc.tile_pool(name="gat", bufs=GAT_BUFS) as gat,
            ):
                # ---- stage 4: gather + select + reduce ----
                iota4 = keep.tile([P, 4], F32)
                nc.sync.dma_start(iota4[:], io4_t[:])
                r2_sb = keep.tile([P, SLOTS], F32)
                nc.sync.dma_start(r2_sb[:], r2_t[:])
                half_sb = keep.tile([P, NBLK], F32)
                out_sb = keep.tile([P, G], F32)
                for q, b0, nb in plan * repeat:
                    n = nb * BLK
                    idxw_sb = gat.tile(
                        [P, MAXB * ICOL], I16, tag="idxw", name=f"idxw{b0}"
                    )
                    nc.sync.dma_start(
                        idxw_sb[:, 0 : nb * ICOL],
                        idxw_t[:, b0 * ICOL : (b0 + nb) * ICOL],
                    )
                    gth = gat.tile(
                        [P, MAXB * BLK, ELEM], F32, tag="gth", name=f"gth{b0}"
                    )
                    dma_gather_raw(
                        nc.gpsimd,
                        gth[:, 0:n, :],
                        S16[:, 0:ELEM],
                        idxw_sb[:, 0 : nb * ICOL],
                        P * n,
                        P * n,
                        elem_size=ELEM,
                        elem_step=64,
                        queue_num=q,
                        single_packet=SINGLE_PACKET,
                    )
                    mask = gat.tile(
                        [P, MAXB * BLK, 4], F32, tag="mask", name=f"mask{b0}"
                    )
                    nc.vector.tensor_tensor(
                        out=mask[:, 0:n, :],
                        in0=r2_sb[:, b0 * BLK : b0 * BLK + n]
                        .unsqueeze(2)
                        .to_broadcast([P, n, 4]),
                        in1=iota4[:].unsqueeze(1).to_broadcast([P, n, 4]),
                        op=mybir.AluOpType.is_equal,
                    )
                    msel = gat.tile(
                        [P, MAXB * BLK, 4], F32, tag="msel", name=f"msel{b0}"
                    )
                    nc.vector.tensor_tensor(
                        out=msel[:, 0:n, :],
                        in0=mask[:, 0:n, :],
                        in1=gth[:, 0:n, 0:4],
                        op=mybir.AluOpType.mult,
                    )
                    nc.vector.tensor_reduce(
                        out=half_sb[:, b0 : b0 + nb].unsqueeze(2),
                        in_=msel[:, 0:n, :]
                        .rearrange("p a b -> p (a b)")
                        .rearrange("p (n x) -> p n x", x=BLK * 4),
                        axis=mybir.AxisListType.X,
                        op=mybir.AluOpType.add,
                    )
                nc.vector.tensor_reduce(
                    out=out_sb[:].unsqueeze(2),
                    in_=half_sb[:].rearrange("p (g h) -> p g h", h=HB),
                    axis=mybir.AxisListType.X,
                    op=mybir.AluOpType.add,
                )
                nc.sync.dma_start(out_t[:], out_sb[:])
    nc.compile()
    return nc


def make_in_maps(word_idxs, embed_table, weights, G, L, D, RPP, CPI, ncores=NCORES):
    """Shard + lay out the full inputs for the per-core program."""
    BPC = G * P
    SLOTS = G * L
    NT = SLOTS // CPI
    VPC = P * RPP
    import ml_dtypes

    bf16 = ml_dtypes.bfloat16
    idx = np.asarray(word_idxs).astype(np.int32)
    tab = np.asarray(embed_table, dtype=np.float32)
    w = np.asarray(weights, dtype=np.float32).reshape(-1)
    V = tab.shape[0]
    tab_pad = np.zeros((VPC * ncores, D), dtype=bf16)
    tab_pad[:V] = tab.astype(bf16)
    w_c = np.ascontiguousarray(
        np.broadcast_to((w / np.float32(L))[None, :].astype(bf16), (P, D))
    )
    in_maps = []
    for c in range(ncores):
        # token slot layout: [partition p, slot j=g*L+l] holds idx of batch
        # row (c*BPC + g*128 + p), token l
        slots = (
            idx[c * BPC : (c + 1) * BPC]
            .reshape(G, P, L)
            .transpose(1, 0, 2)
            .reshape(P, SLOTS)
        )
        jmat = (slots >> 2).astype(np.int16)  # [P, SLOTS]
        r2 = (slots & 3).astype(np.float32)
        # per-instruction index lists in i = c_local*128 + p order, wrapped
        # into the dma_gather [16, NI//16] layout, replicated to 128 parts
        u = jmat.reshape(P, NT, CPI).transpose(1, 2, 0)  # [NT, CPI, P]
        wrp = u.reshape(NT, CPI * P // 16, 16).transpose(2, 0, 1).reshape(16, -1)
        idxw = np.ascontiguousarray(np.tile(wrp, (8, 1)))  # [128, SLOTS*8]
        tab_c = np.ascontiguousarray(
            tab_pad[c * VPC : (c + 1) * VPC].reshape(P, RPP * D)
        )
        in_maps.append(
            {
                "idxw": idxw,
                "r2": np.ascontiguousarray(r2),
                "tab": tab_c,
                "w": w_c,
                "io4": np.ascontiguousarray(
                    np.broadcast_to(np.arange(4, dtype=np.float32), (P, 4))
                ),
            }
        )
    return in_maps


def unshard_out(results, G, ncores=NCORES):
    """results: list of per-core {'out': [128, G]} -> full [B, 1] f32."""
    parts = []
    for c in range(ncores):
        o = np.asarray(results[c]["out"])  # [P, G]; out[p, g] = row g*128+p
        parts.append(o.T.reshape(-1))
    return np.concatenate(parts).reshape(-1, 1).astype(np.float32)


_CACHED_NC = None

FULL = dict(G=16, L=200, D=100, RPP=98, CPI=25)


def _get_nc():
    global _CACHED_NC
    if _CACHED_NC is None:
        _CACHED_NC = build_program(**FULL)
    return _CACHED_NC


def run(word_idxs, embed_table, weights, trace=False, **spmd_kwargs):
    """Build (cached), run on the 8 cores, return (full_out, BassKernelResults)."""
    nc = _get_nc()
    in_maps = make_in_maps(
        word_idxs,
        embed_table,
        weights,
        FULL["G"],
        FULL["L"],
        FULL["D"],
        FULL["RPP"],
        FULL["CPI"],
    )
    res = run_bass_kernel_spmd(
        nc, in_maps, core_ids=list(range(NCORES)), trace=trace, **spmd_kwargs
    )
    out = unshard_out(res.results, FULL["G"])
    return out, res


def kernel(word_idxs, embed_table, weights):
    out, _ = run(word_idxs, embed_table, weights, trace=False)
    return out



# revision 24
# speedup vs baseline: 2.0006x; 1.0147x over previous
"""Trainium2 Bass kernel for fused embedding-lookup -> mean-pool -> dot(weights).

Reference computation (B=16384, L=200, D=100, V=100000):
    out[b] = mean_l(embed_table[word_idxs[b, l], :]) @ weights            # [B, 1]

Key algebraic transform: the dot with `weights` is linear, so
    out[b] = sum_l s[word_idxs[b, l]],   with  s = embed_table @ (weights / L)
Instead of gathering B*L rows of 400B (1.31 GB), we precompute the V-element
vector `s` on-device (the 40MB table is read exactly once across the 8 cores)
and gather B*L scalars.

The scalar gather uses the TIE-ucode `dma_gather` (int16 row indices). Its
row stride must be a 256B multiple, so s is spread into a bf16 table
    S16[j, 0:4] = s_pad[4j .. 4j+4),  row pitch 128 bf16 = 256B,
and a token with index v gathers row j = v >> 2 (8B payload), after which a
4-wide mask+reduce selects lane r = v & 3 and accumulates per row-group.

Performance structure (HW-measured): the wall is Q7 descriptor GENERATION
inside the dma_gather ucode — each gather runs on one Q7 core pair (pair =
queue_num) at ~9.4 cycles/descriptor, one descriptor per token. All four
queue pairs generate concurrently (the 8 Q7 cores pop the NX instruction
queue asynchronously), so gathers are issued as four per-queue chains with
STAGGERED first/last instruction sizes (1/2/3/4 blocks of 25 token-slots):
equal-sized chains fall into drain lockstep and idle ~20% of the time.
Per-core: 409600 tokens at ~2.2ns/token effective -> ~0.9ms gather phase.

Sharding (8 cores): batch-parallel gather (2048 rows/core); vocab-parallel s
precompute in bf16 (12544 padded rows/core) + AllGather, then a one-shot
SBUF-built spread + single dense 3.2MB DRAM write (never per-row
descriptors). First-wave index tiles are prefetched ahead of stage 1 so the
in-order Sync engine has them loaded before the spread table lands.

Host does layout only: shard/reshape inputs, compute j = idx>>2 / r = idx&3,
wrap indices in the dma_gather [16, S] layout, and concat per-core outputs.
"""

import os
import sys

import numpy as np

for _p in ("/opt/trn_rl_repo",):
    if os.path.isdir(_p) and _p not in sys.path:
        sys.path.insert(0, _p)

from concourse import bacc, bass, mybir, tile  # noqa: E402
from concourse.bass_utils import run_bass_kernel_spmd  # noqa: E402

F32 = mybir.dt.float32
BF16 = mybir.dt.bfloat16
I32 = mybir.dt.int32
I16 = mybir.dt.int16
P = 128
NCORES = 8


def dma_gather_raw(
    gp, out_ap, in_ap, idxs_ap, num_idxs, num_idxs_reg, elem_size, elem_step,
    queue_num=0, single_packet=False,
):
    """nc.gpsimd.dma_gather minus the 256B *element* restriction.

    Only the source row PITCH must be a 256B multiple (stride_bytes_256 is an
    8-bit field in 256B units); the per-index element payload can be smaller.
    Emits the same InstDMAGatherAnt the stock wrapper does.
    """
    dt_sz = mybir.dt.size(in_ap.dtype)
    stride_256 = (elem_step * dt_sz) // 256
    assert elem_step * dt_sz == stride_256 * 256 and 0 < stride_256 < 256
    assert in_ap.ap[0][0] == elem_step and in_ap.ap[-1][1] == elem_size
    _in_ap = gp.lower_ap_dma(in_ap, for_custom_bir_dma=True)
    _idxs_ap = gp.lower_ap(idxs_ap)
    _out_ap = gp.lower_ap(out_ap)
    return gp.add_instruction(
        mybir.InstDMAGatherAnt(
            name=gp.bass.get_next_instruction_name(),
            ins=[*_in_ap, _idxs_ap, gp.lower_val_access(gp.to_reg(num_idxs_reg))],
            outs=[_out_ap],
            transpose=False,
            num_idxs=num_idxs,
            elem_size=elem_size,
            stride_bytes_256=stride_256,
            gen_mode=0,
            single_packet=single_packet,
            queue_num=queue_num,
            sbuf_tokens_per_rank=0,
            sbuf_free_dim_per_rank=0,
            sbuf_free_dim_pad_per_rank=0,
            sbuf_byte_offset=0,
        )
    )


def build_program(
    G=16, L=200, D=100, RPP=98, CPI=100, NQ=4, ncores=NCORES, use_collective=True,
    repeat=1, ELEM=4, GAT_BUFS=8, SINGLE_PACKET=False,
):
    """Build the SPMD program (identical on all cores).

    G:   row-groups per core (batch rows per core = G*128)
    L:   tokens per row
    D:   embedding dim
    RPP: padded vocab rows per SBUF partition (vocab rows per core = 128*RPP)
    CPI: token-slot BLOCK size (gather instructions cover 1..4 blocks);
         L % CPI == 0
    NQ:  SWDGE queues to rotate over (1..4)

    Gather instructions are grouped into per-queue chains whose first (and
    last) instructions have staggered sizes (1/2/3/4 blocks). Same-queue
    instructions serialize through descriptor-ring space, so equal-sized
    chains would fall into lockstep: all four queues' SDMA drains collide
    after each descgen wave and every queue idles through the combined
    drain. Staggered chain heads phase-shift the queues so each queue's
    drain overlaps the other queues' descgen.
    """
    assert L % CPI == 0
    BLK = CPI  # token slots per block
    SLOTS = G * L  # token slots per partition
    NBLK = SLOTS // BLK  # total blocks
    HB = L // BLK  # blocks per row-group
    ICOL = P * BLK // 16  # idxw columns per block
    MAXB = 4  # max blocks per gather instruction
    # per-queue chains in blocks: [q+1, 4, 4, ..., remainder]
    base = NBLK // NQ
    plan = []  # (queue, first block, nblocks), in issue order
    chains = []
    for q in range(NQ):
        first = min(q + 1, base)
        rest = base - first
        sizes = [first] + [MAXB] * (rest // MAXB)
        if rest % MAXB:
            sizes.append(rest % MAXB)
        chains.append(sizes)
    blk0 = 0
    pos = [0] * NQ
    while any(pos[q] < len(chains[q]) for q in range(NQ)):
        for q in range(NQ):
            if pos[q] < len(chains[q]):
                n = chains[q][pos[q]]
                pos[q] += 1
                plan.append((q, blk0, n))
                blk0 += n
    assert blk0 == NBLK
    VPC = P * RPP
    V_PAD = VPC * ncores
    NROWS = V_PAD // 4  # row j holds s[4j .. 4j+4)
    nc = bacc.Bacc(
        "TRN2",
        target_bir_lowering=False,
        debug=False,
        num_devices=ncores,
        num_swdge_queues=NQ,
    )
    idxw_t = nc.dram_tensor("idxw", [P, SLOTS * 8], I16, kind="ExternalInput")
    io4_t = nc.dram_tensor("io4", [P, 4], BF16, kind="ExternalInput")
    r2_t = nc.dram_tensor("r2", [P, SLOTS], BF16, kind="ExternalInput")
    tab_t = nc.dram_tensor("tab", [P, RPP * D], BF16, kind="ExternalInput")
    w_t = nc.dram_tensor("w", [P, D], BF16, kind="ExternalInput")
    out_t = nc.dram_tensor("out", [P, G], F32, kind="ExternalOutput")

    with tile.TileContext(nc) as tc:
        with (
            tc.tile_pool(name="dr", bufs=1, space="DRAM") as dr,
            tc.tile_pool(name="keep", bufs=1) as keep,
        ):
            # ---- stage 0: prefetch gather-side inputs ----
            # Emitted before stage 1 so the in-order Sync engine issues these
            # small loads first; otherwise the first wave's idxw sits queued
            # behind the S16 spread write and delays the first gather ~15us.
            iota4 = keep.tile([P, 4], BF16)
            nc.sync.dma_start(iota4[:], io4_t[:])
            r2_sb = keep.tile([P, SLOTS], BF16)
            nc.sync.dma_start(r2_sb[:], r2_t[:])
            first_idxw = {}
            for q, b0, nb in plan[: NQ]:
                t_ = keep.tile([P, MAXB * ICOL], I16, name=f"idxwf{b0}")
                nc.sync.dma_start(
                    t_[:, 0 : nb * ICOL],
                    idxw_t[:, b0 * ICOL : (b0 + nb) * ICOL],
                )
                first_idxw[b0] = t_
            half_sb = keep.tile([P, NBLK], F32)
            out_sb = keep.tile([P, G], F32)

            with tc.tile_pool(name="pre", bufs=1) as pre:
                # ---- stage 1: s_part = (table slice) @ (w/L) ----
                # bf16 table/weights: halves the 5MB load and doubles DVE
                # throughput for the product; the reduce accumulates to f32.
                tab_sb = pre.tile([P, RPP * D], BF16)
                nc.sync.dma_start(tab_sb[:], tab_t[:])
                w_sb = pre.tile([P, D], BF16)
                nc.sync.dma_start(w_sb[:], w_t[:])
                prod_sb = pre.tile([P, RPP * D], BF16)
                nc.vector.tensor_tensor(
                    out=prod_sb[:].rearrange("p (r d) -> p r d", d=D),
                    in0=tab_sb[:].rearrange("p (r d) -> p r d", d=D),
                    in1=w_sb[:].unsqueeze(1).to_broadcast([P, RPP, D]),
                    op=mybir.AluOpType.mult,
                )
                s_sb = pre.tile([P, RPP], F32)
                nc.vector.tensor_reduce(
                    out=s_sb[:].unsqueeze(2),
                    in_=prod_sb[:].rearrange("p (r d) -> p r d", d=D),
                    axis=mybir.AxisListType.X,
                    op=mybir.AluOpType.add,
                )

                # ---- stage 2: AllGather s ----
                s_part = dr.tile([P, RPP], F32)
                nc.sync.dma_start(s_part[:], s_sb[:])
                if use_collective:
                    s_full = dr.tile([ncores * RPP, P], F32, addr_space="Shared")
                    nc.gpsimd.collective_compute(
                        "AllGather",
                        mybir.AluOpType.bypass,
                        replica_groups=[list(range(ncores))],
                        ins=[s_part.opt()],
                        outs=[s_full.opt()],
                    )
                else:
                    # crash-isolation mode: fill s_full with the local part
                    # replicated (wrong data, same program shape)
                    s_full = dr.tile([ncores * RPP, P], F32)
                    for c in range(ncores):
                        nc.sync.dma_start(
                            s_full[c * RPP : (c + 1) * RPP, :],
                            s_part[:].rearrange("p r -> (p r)").rearrange(
                                "(r q) -> r q", q=P
                            ),
                        )

                # ---- stage 3: spread s into 256B-pitch rows ----
                # S16[j, 0:4] = s[4j .. 4j+4) in bf16; rows pitched 128 bf16
                # = 256B so the gather row stride is ISA-encodable; lanes
                # 4..127 are never written nor read. Token v -> row v>>2,
                # lane v&3.
                #
                # Build the spread layout in SBUF with one strided DVE
                # cast-copy, then ship it to DRAM as a single dense 3.2MB
                # transfer. (The per-row descriptor route — 25088 16B
                # descriptors — costs ~69us of HWDGE/SDMA time.)
                RPP64 = NROWS // P  # spread rows per partition
                S16 = dr.tile([NROWS, 128], BF16)
                sfull_sb = pre.tile([P, RPP64 * 4], F32)
                nc.sync.dma_start(
                    sfull_sb[:],
                    s_full[:]
                    .rearrange("a b -> (a b)")
                    .rearrange("(p x) -> p x", p=P),
                )
                ssp_sb = pre.tile([P, RPP64 * 128], BF16)
                # lanes 4.. of each 256B row are never read by the gather
                # (in_ap covers lanes 0:ELEM only), so they ship to DRAM
                # uninitialized — no memset needed.
                nc.vector.tensor_copy(
                    out=ssp_sb[:].rearrange("p (r k) -> p r k", k=128)[:, :, 0:4],
                    in_=sfull_sb[:].rearrange("p (r q) -> p r q", q=4),
                )
                nc.sync.dma_start(
                    S16[:].rearrange("a b -> (a b)").rearrange("(p x) -> p x", p=P),
                    ssp_sb[:],
                )

            with tc.tile_pool(name="gat", bufs=GAT_BUFS) as gat:
                # ---- stage 4: gather + select + reduce ----
                for i, (q, b0, nb) in enumerate(plan * repeat):
                    n = nb * BLK
                    if i < NQ:
                        idxw_sb = first_idxw[b0]
                    else:
                        idxw_sb = gat.tile(
                            [P, MAXB * ICOL], I16, tag="idxw", name=f"idxw{b0}"
                        )
                        nc.sync.dma_start(
                            idxw_sb[:, 0 : nb * ICOL],
                            idxw_t[:, b0 * ICOL : (b0 + nb) * ICOL],
                        )
                    gth = gat.tile(
                        [P, MAXB * BLK, ELEM], BF16, tag="gth", name=f"gth{b0}"
                    )
                    dma_gather_raw(
                        nc.gpsimd,
                        gth[:, 0:n, :],
                        S16[:, 0:ELEM],
                        idxw_sb[:, 0 : nb * ICOL],
                        P * n,
                        P * n,
                        elem_size=ELEM,
                        elem_step=128,
                        queue_num=q,
                        single_packet=SINGLE_PACKET,
                    )
                    mask = gat.tile(
                        [P, MAXB * BLK, 4], BF16, tag="mask", name=f"mask{b0}"
                    )
                    nc.vector.tensor_tensor(
                        out=mask[:, 0:n, :],
                        in0=r2_sb[:, b0 * BLK : b0 * BLK + n]
                        .unsqueeze(2)
                        .to_broadcast([P, n, 4]),
                        in1=iota4[:].unsqueeze(1).to_broadcast([P, n, 4]),
                        op=mybir.AluOpType.is_equal,
                    )
                    msel = gat.tile(
                        [P, MAXB * BLK, 4], BF16, tag="msel", name=f"msel{b0}"
                    )
                    nc.vector.tensor_tensor(
                        out=msel[:, 0:n, :],
                        in0=mask[:, 0:n, :],
                        in1=gth[:, 0:n, 0:4],
                        op=mybir.AluOpType.mult,
                    )
                    nc.vector.tensor_reduce(
                        out=half_sb[:, b0 : b0 + nb].unsqueeze(2),
                        in_=msel[:, 0:n, :]
                        .rearrange("p a b -> p (a b)")
                        .rearrange("p (n x) -> p n x", x=BLK * 4),
                        axis=mybir.AxisListType.X,
                        op=mybir.AluOpType.add,
                    )
                nc.vector.tensor_reduce(
                    out=out_sb[:].unsqueeze(2),
                    in_=half_sb[:].rearrange("p (g h) -> p g h", h=HB),
                    axis=mybir.AxisListType.X,
                    op=mybir.AluOpType.add,
                )
                nc.sync.dma_start(out_t[:], out_sb[:])
    nc.compile()
    return nc


def make_in_maps(word_idxs, embed_table, weights, G, L, D, RPP, CPI, ncores=NCORES):
    """Shard + lay out the full inputs for the per-core program."""
    BPC = G * P
    SLOTS = G * L
    NT = SLOTS // CPI
    VPC = P * RPP
    import ml_dtypes

    bf16 = ml_dtypes.bfloat16
    idx = np.asarray(word_idxs).astype(np.int32)
    tab = np.asarray(embed_table, dtype=np.float32)
    w = np.asarray(weights, dtype=np.float32).reshape(-1)
    V = tab.shape[0]
    tab_pad = np.zeros((VPC * ncores, D), dtype=bf16)
    tab_pad[:V] = tab.astype(bf16)
    w_c = np.ascontiguousarray(
        np.broadcast_to((w / np.float32(L))[None, :].astype(bf16), (P, D))
    )
    in_maps = []
    for c in range(ncores):
        # token slot layout: [partition p, slot j=g*L+l] holds idx of batch
        # row (c*BPC + g*128 + p), token l
        slots = (
            idx[c * BPC : (c + 1) * BPC]
            .reshape(G, P, L)
            .transpose(1, 0, 2)
            .reshape(P, SLOTS)
        )
        jmat = (slots >> 2).astype(np.int16)  # [P, SLOTS]
        r2 = (slots & 3).astype(bf16)
        # per-instruction index lists in i = c_local*128 + p order, wrapped
        # into the dma_gather [16, NI//16] layout, replicated to 128 parts
        u = jmat.reshape(P, NT, CPI).transpose(1, 2, 0)  # [NT, CPI, P]
        wrp = u.reshape(NT, CPI * P // 16, 16).transpose(2, 0, 1).reshape(16, -1)
        idxw = np.ascontiguousarray(np.tile(wrp, (8, 1)))  # [128, SLOTS*8]
        tab_c = np.ascontiguousarray(
            tab_pad[c * VPC : (c + 1) * VPC].reshape(P, RPP * D)
        )
        in_maps.append(
            {
                "idxw": idxw,
                "r2": np.ascontiguousarray(r2),
                "tab": tab_c,
                "w": w_c,
                "io4": np.ascontiguousarray(
                    np.broadcast_to(np.arange(4, dtype=np.float32).astype(bf16), (P, 4))
                ),
            }
        )
    return in_maps


def unshard_out(results, G, ncores=NCORES):
    """results: list of per-core {'out': [128, G]} -> full [B, 1] f32."""
    parts = []
    for c in range(ncores):
        o = np.asarray(results[c]["out"])  # [P, G]; out[p, g] = row g*128+p
        parts.append(o.T.reshape(-1))
    return np.concatenate(parts).reshape(-1, 1).astype(np.float32)


_CACHED_NC = None

FULL = dict(G=16, L=200, D=100, RPP=98, CPI=25)


def _get_nc():
    global _CACHED_NC
    if _CACHED_NC is None:
        _CACHED_NC = build_program(**FULL)
    return _CACHED_NC


def run(word_idxs, embed_table, weights, trace=False, **spmd_kwargs):
    """Build (cached), run on the 8 cores, return (full_out, BassKernelResults)."""
    nc = _get_nc()
    in_maps = make_in_maps(
        word_idxs,
        embed_table,
        weights,
        FULL["G"],
        FULL["L"],
        FULL["D"],
        FULL["RPP"],
        FULL["CPI"],
    )
    res = run_bass_kernel_spmd(
        nc, in_maps, core_ids=list(range(NCORES)), trace=trace, **spmd_kwargs
    )
    out = unshard_out(res.results, FULL["G"])
    return out, res


def kernel(word_idxs, embed_table, weights):
    out, _ = run(word_idxs, embed_table, weights, trace=False)
    return out



# revision 25
# speedup vs baseline: 2.1723x; 1.0859x over previous
"""Trainium2 Bass kernel for fused embedding-lookup -> mean-pool -> dot(weights).

Reference computation (B=16384, L=200, D=100, V=100000):
    out[b] = mean_l(embed_table[word_idxs[b, l], :]) @ weights            # [B, 1]

Key algebraic transform: the dot with `weights` is linear, so
    out[b] = sum_l s[word_idxs[b, l]],   with  s = embed_table @ (weights / L)
Instead of gathering B*L rows of 400B (1.31 GB), we precompute the V-element
vector `s` on-device (the 40MB table is read exactly once across the 8 cores)
and gather B*L scalars.

The scalar gather uses the TIE-ucode `dma_gather` (int16 row indices). Its
row stride must be a 256B multiple, so s is spread into a bf16 table
    S16[j, 0:4] = s_pad[4j .. 4j+4),  row pitch 128 bf16 = 256B,
and a token with index v gathers row j = v >> 2 (8B payload), after which a
4-wide mask+reduce selects lane r = v & 3 and accumulates per row-group.

Performance structure (HW-measured): the wall is Q7 descriptor GENERATION
inside the dma_gather ucode — each gather runs on one Q7 core pair (pair =
queue_num) at ~9.4 cycles/descriptor, one descriptor per token. All four
queue pairs generate concurrently (the 8 Q7 cores pop the NX instruction
queue asynchronously), so gathers are issued as four per-queue chains with
STAGGERED first/last instruction sizes (1/2/3/4 blocks of 25 token-slots):
equal-sized chains fall into drain lockstep and idle ~20% of the time.
Per-core: 409600 tokens at ~2.2ns/token effective -> ~0.9ms gather phase.

Sharding (8 cores): batch-parallel gather (2048 rows/core); vocab-parallel s
precompute in bf16 (12544 padded rows/core) + AllGather, then a one-shot
SBUF-built spread + single dense 3.2MB DRAM write (never per-row
descriptors). First-wave index tiles are prefetched ahead of stage 1 so the
in-order Sync engine has them loaded before the spread table lands.

Host does layout only: shard/reshape inputs, compute j = idx>>2 / r = idx&3,
wrap indices in the dma_gather [16, S] layout, and concat per-core outputs.
"""

import os
import sys

import numpy as np

for _p in ("/opt/trn_rl_repo",):
    if os.path.isdir(_p) and _p not in sys.path:
        sys.path.insert(0, _p)

from concourse import bacc, bass, mybir, tile  # noqa: E402
from concourse.bass_utils import run_bass_kernel_spmd  # noqa: E402

F32 = mybir.dt.float32
BF16 = mybir.dt.bfloat16
I32 = mybir.dt.int32
I16 = mybir.dt.int16
P = 128
NCORES = 8


def dma_gather_raw(
    gp, out_ap, in_ap, idxs_ap, num_idxs, num_idxs_reg, elem_size, elem_step,
    queue_num=0, single_packet=False,
):
    """nc.gpsimd.dma_gather minus the 256B *element* restriction.

    Only the source row PITCH must be a 256B multiple (stride_bytes_256 is an
    8-bit field in 256B units); the per-index element payload can be smaller.
    Emits the same InstDMAGatherAnt the stock wrapper does.
    """
    dt_sz = mybir.dt.size(in_ap.dtype)
    stride_256 = (elem_step * dt_sz) // 256
    assert elem_step * dt_sz == stride_256 * 256 and 0 < stride_256 < 256
    assert in_ap.ap[0][0] == elem_step and in_ap.ap[-1][1] == elem_size
    _in_ap = gp.lower_ap_dma(in_ap, for_custom_bir_dma=True)
    _idxs_ap = gp.lower_ap(idxs_ap)
    _out_ap = gp.lower_ap(out_ap)
    return gp.add_instruction(
        mybir.InstDMAGatherAnt(
            name=gp.bass.get_next_instruction_name(),
            ins=[*_in_ap, _idxs_ap, gp.lower_val_access(gp.to_reg(num_idxs_reg))],
            outs=[_out_ap],
            transpose=False,
            num_idxs=num_idxs,
            elem_size=elem_size,
            stride_bytes_256=stride_256,
            gen_mode=0,
            single_packet=single_packet,
            queue_num=queue_num,
            sbuf_tokens_per_rank=0,
            sbuf_free_dim_per_rank=0,
            sbuf_free_dim_pad_per_rank=0,
            sbuf_byte_offset=0,
        )
    )


def build_program(
    G=16, L=200, D=100, RPP=98, CPI=100, NQ=4, ncores=NCORES, use_collective=True,
    repeat=1, ELEM=4, GAT_BUFS=8, SINGLE_PACKET=False,
):
    """Build the SPMD program (identical on all cores).

    G:   row-groups per core (batch rows per core = G*128)
    L:   tokens per row
    D:   embedding dim
    RPP: padded vocab rows per SBUF partition (vocab rows per core = 128*RPP)
    CPI: token-slot BLOCK size (gather instructions cover 1..4 blocks);
         L % CPI == 0
    NQ:  SWDGE queues to rotate over (1..4)

    Gather instructions are grouped into per-queue chains whose first (and
    last) instructions have staggered sizes (1/2/3/4 blocks). Same-queue
    instructions serialize through descriptor-ring space, so equal-sized
    chains would fall into lockstep: all four queues' SDMA drains collide
    after each descgen wave and every queue idles through the combined
    drain. Staggered chain heads phase-shift the queues so each queue's
    drain overlaps the other queues' descgen.
    """
    assert L % CPI == 0
    BLK = CPI  # token slots per block
    SLOTS = G * L  # token slots per partition
    NBLK = SLOTS // BLK  # total blocks
    HB = L // BLK  # blocks per row-group
    ICOL = P * BLK // 16  # idxw columns per block
    MAXB = 2  # max blocks per gather instruction
    # Per-queue chains in blocks. Small instructions keep each queue's
    # descriptor ring holding >1 instruction, so Q7 descgen runs
    # back-to-back instead of stalling on the NX-decode ring-space await
    # (which also head-of-line blocks later queues' decode). Odd first
    # sizes phase-shift half the queues by one block.
    base = NBLK // NQ
    plan = []  # (queue, first block, nblocks), in issue order
    chains = []
    for q in range(NQ):
        first = 1 if q % 2 == 0 else MAXB
        rest = base - first
        sizes = [first] + [MAXB] * (rest // MAXB)
        if rest % MAXB:
            sizes.append(rest % MAXB)
        chains.append(sizes)
    blk0 = 0
    pos = [0] * NQ
    while any(pos[q] < len(chains[q]) for q in range(NQ)):
        for q in range(NQ):
            if pos[q] < len(chains[q]):
                n = chains[q][pos[q]]
                pos[q] += 1
                plan.append((q, blk0, n))
                blk0 += n
    assert blk0 == NBLK
    VPC = P * RPP
    V_PAD = VPC * ncores
    NROWS = V_PAD // 4  # row j holds s[4j .. 4j+4)
    nc = bacc.Bacc(
        "TRN2",
        target_bir_lowering=False,
        debug=False,
        num_devices=ncores,
        num_swdge_queues=NQ,
    )
    idxw_t = nc.dram_tensor("idxw", [P, SLOTS * 8], I16, kind="ExternalInput")
    io4_t = nc.dram_tensor("io4", [P, 4], BF16, kind="ExternalInput")
    r2_t = nc.dram_tensor("r2", [P, SLOTS], BF16, kind="ExternalInput")
    tab_t = nc.dram_tensor("tab", [P, RPP * D], BF16, kind="ExternalInput")
    w_t = nc.dram_tensor("w", [P, D], BF16, kind="ExternalInput")
    out_t = nc.dram_tensor("out", [P, G], F32, kind="ExternalOutput")

    with tile.TileContext(nc) as tc:
        with (
            tc.tile_pool(name="dr", bufs=1, space="DRAM") as dr,
            tc.tile_pool(name="keep", bufs=1) as keep,
        ):
            # ---- stage 0: prefetch gather-side inputs ----
            # Emitted before stage 1 so the in-order Sync engine issues these
            # small loads first; otherwise the first wave's idxw sits queued
            # behind the S16 spread write and delays the first gather ~15us.
            iota4 = keep.tile([P, 4], BF16)
            nc.sync.dma_start(iota4[:], io4_t[:])
            r2_sb = keep.tile([P, SLOTS], BF16)
            nc.sync.dma_start(r2_sb[:], r2_t[:])
            first_idxw = {}
            for q, b0, nb in plan[: NQ]:
                t_ = keep.tile([P, MAXB * ICOL], I16, name=f"idxwf{b0}")
                nc.sync.dma_start(
                    t_[:, 0 : nb * ICOL],
                    idxw_t[:, b0 * ICOL : (b0 + nb) * ICOL],
                )
                first_idxw[b0] = t_
            half_sb = keep.tile([P, NBLK], F32)
            out_sb = keep.tile([P, G], F32)

            with tc.tile_pool(name="pre", bufs=1) as pre:
                # ---- stage 1: s_part = (table slice) @ (w/L) ----
                # bf16 table/weights: halves the 5MB load and doubles DVE
                # throughput for the product; the reduce accumulates to f32.
                tab_sb = pre.tile([P, RPP * D], BF16)
                nc.sync.dma_start(tab_sb[:], tab_t[:])
                w_sb = pre.tile([P, D], BF16)
                nc.sync.dma_start(w_sb[:], w_t[:])
                prod_sb = pre.tile([P, RPP * D], BF16)
                nc.vector.tensor_tensor(
                    out=prod_sb[:].rearrange("p (r d) -> p r d", d=D),
                    in0=tab_sb[:].rearrange("p (r d) -> p r d", d=D),
                    in1=w_sb[:].unsqueeze(1).to_broadcast([P, RPP, D]),
                    op=mybir.AluOpType.mult,
                )
                s_sb = pre.tile([P, RPP], F32)
                nc.vector.tensor_reduce(
                    out=s_sb[:].unsqueeze(2),
                    in_=prod_sb[:].rearrange("p (r d) -> p r d", d=D),
                    axis=mybir.AxisListType.X,
                    op=mybir.AluOpType.add,
                )

                # ---- stage 2: AllGather s ----
                s_part = dr.tile([P, RPP], F32)
                nc.sync.dma_start(s_part[:], s_sb[:])
                if use_collective:
                    s_full = dr.tile([ncores * RPP, P], F32, addr_space="Shared")
                    nc.gpsimd.collective_compute(
                        "AllGather",
                        mybir.AluOpType.bypass,
                        replica_groups=[list(range(ncores))],
                        ins=[s_part.opt()],
                        outs=[s_full.opt()],
                    )
                else:
                    # crash-isolation mode: fill s_full with the local part
                    # replicated (wrong data, same program shape)
                    s_full = dr.tile([ncores * RPP, P], F32)
                    for c in range(ncores):
                        nc.sync.dma_start(
                            s_full[c * RPP : (c + 1) * RPP, :],
                            s_part[:].rearrange("p r -> (p r)").rearrange(
                                "(r q) -> r q", q=P
                            ),
                        )

                # ---- stage 3: spread s into 256B-pitch rows ----
                # S16[j, 0:4] = s[4j .. 4j+4) in bf16; rows pitched 128 bf16
                # = 256B so the gather row stride is ISA-encodable; lanes
                # 4..127 are never written nor read. Token v -> row v>>2,
                # lane v&3.
                #
                # Build the spread layout in SBUF with one strided DVE
                # cast-copy, then ship it to DRAM as a single dense 3.2MB
                # transfer. (The per-row descriptor route — 25088 16B
                # descriptors — costs ~69us of HWDGE/SDMA time.)
                RPP64 = NROWS // P  # spread rows per partition
                S16 = dr.tile([NROWS, 128], BF16)
                sfull_sb = pre.tile([P, RPP64 * 4], F32)
                nc.sync.dma_start(
                    sfull_sb[:],
                    s_full[:]
                    .rearrange("a b -> (a b)")
                    .rearrange("(p x) -> p x", p=P),
                )
                ssp_sb = pre.tile([P, RPP64 * 128], BF16)
                # lanes 4.. of each 256B row are never read by the gather
                # (in_ap covers lanes 0:ELEM only), so they ship to DRAM
                # uninitialized — no memset needed.
                nc.vector.tensor_copy(
                    out=ssp_sb[:].rearrange("p (r k) -> p r k", k=128)[:, :, 0:4],
                    in_=sfull_sb[:].rearrange("p (r q) -> p r q", q=4),
                )
                nc.sync.dma_start(
                    S16[:].rearrange("a b -> (a b)").rearrange("(p x) -> p x", p=P),
                    ssp_sb[:],
                )

            with tc.tile_pool(name="gat", bufs=GAT_BUFS) as gat:
                # ---- stage 4: gather + select + reduce ----
                for i, (q, b0, nb) in enumerate(plan * repeat):
                    n = nb * BLK
                    if i < NQ:
                        idxw_sb = first_idxw[b0]
                    else:
                        idxw_sb = gat.tile(
                            [P, MAXB * ICOL], I16, tag="idxw", name=f"idxw{b0}"
                        )
                        nc.sync.dma_start(
                            idxw_sb[:, 0 : nb * ICOL],
                            idxw_t[:, b0 * ICOL : (b0 + nb) * ICOL],
                        )
                    gth = gat.tile(
                        [P, MAXB * BLK, ELEM], BF16, tag="gth", name=f"gth{b0}"
                    )
                    dma_gather_raw(
                        nc.gpsimd,
                        gth[:, 0:n, :],
                        S16[:, 0:ELEM],
                        idxw_sb[:, 0 : nb * ICOL],
                        P * n,
                        P * n,
                        elem_size=ELEM,
                        elem_step=128,
                        queue_num=q,
                        single_packet=SINGLE_PACKET,
                    )
                    mask = gat.tile(
                        [P, MAXB * BLK, 4], BF16, tag="mask", name=f"mask{b0}"
                    )
                    nc.vector.tensor_tensor(
                        out=mask[:, 0:n, :],
                        in0=r2_sb[:, b0 * BLK : b0 * BLK + n]
                        .unsqueeze(2)
                        .to_broadcast([P, n, 4]),
                        in1=iota4[:].unsqueeze(1).to_broadcast([P, n, 4]),
                        op=mybir.AluOpType.is_equal,
                    )
                    msel = gat.tile(
                        [P, MAXB * BLK, 4], BF16, tag="msel", name=f"msel{b0}"
                    )
                    nc.vector.tensor_tensor(
                        out=msel[:, 0:n, :],
                        in0=mask[:, 0:n, :],
                        in1=gth[:, 0:n, 0:4],
                        op=mybir.AluOpType.mult,
                    )
                    nc.vector.tensor_reduce(
                        out=half_sb[:, b0 : b0 + nb].unsqueeze(2),
                        in_=msel[:, 0:n, :]
                        .rearrange("p a b -> p (a b)")
                        .rearrange("p (n x) -> p n x", x=BLK * 4),
                        axis=mybir.AxisListType.X,
                        op=mybir.AluOpType.add,
                    )
                nc.vector.tensor_reduce(
                    out=out_sb[:].unsqueeze(2),
                    in_=half_sb[:].rearrange("p (g h) -> p g h", h=HB),
                    axis=mybir.AxisListType.X,
                    op=mybir.AluOpType.add,
                )
                nc.sync.dma_start(out_t[:], out_sb[:])
    nc.compile()
    return nc


def make_in_maps(word_idxs, embed_table, weights, G, L, D, RPP, CPI, ncores=NCORES):
    """Shard + lay out the full inputs for the per-core program."""
    BPC = G * P
    SLOTS = G * L
    NT = SLOTS // CPI
    VPC = P * RPP
    import ml_dtypes

    bf16 = ml_dtypes.bfloat16
    idx = np.asarray(word_idxs).astype(np.int32)
    tab = np.asarray(embed_table, dtype=np.float32)
    w = np.asarray(weights, dtype=np.float32).reshape(-1)
    V = tab.shape[0]
    tab_pad = np.zeros((VPC * ncores, D), dtype=bf16)
    tab_pad[:V] = tab.astype(bf16)
    w_c = np.ascontiguousarray(
        np.broadcast_to((w / np.float32(L))[None, :].astype(bf16), (P, D))
    )
    in_maps = []
    for c in range(ncores):
        # token slot layout: [partition p, slot j=g*L+l] holds idx of batch
        # row (c*BPC + g*128 + p), token l
        slots = (
            idx[c * BPC : (c + 1) * BPC]
            .reshape(G, P, L)
            .transpose(1, 0, 2)
            .reshape(P, SLOTS)
        )
        jmat = (slots >> 2).astype(np.int16)  # [P, SLOTS]
        r2 = (slots & 3).astype(bf16)
        # per-instruction index lists in i = c_local*128 + p order, wrapped
        # into the dma_gather [16, NI//16] layout, replicated to 128 parts
        u = jmat.reshape(P, NT, CPI).transpose(1, 2, 0)  # [NT, CPI, P]
        wrp = u.reshape(NT, CPI * P // 16, 16).transpose(2, 0, 1).reshape(16, -1)
        idxw = np.ascontiguousarray(np.tile(wrp, (8, 1)))  # [128, SLOTS*8]
        tab_c = np.ascontiguousarray(
            tab_pad[c * VPC : (c + 1) * VPC].reshape(P, RPP * D)
        )
        in_maps.append(
            {
                "idxw": idxw,
                "r2": np.ascontiguousarray(r2),
                "tab": tab_c,
                "w": w_c,
                "io4": np.ascontiguousarray(
                    np.broadcast_to(np.arange(4, dtype=np.float32).astype(bf16), (P, 4))
                ),
            }
        )
    return in_maps


def unshard_out(results, G, ncores=NCORES):
    """results: list of per-core {'out': [128, G]} -> full [B, 1] f32."""
    parts = []
    for c in range(ncores):
        o = np.asarray(results[c]["out"])  # [P, G]; out[p, g] = row g*128+p
        parts.append(o.T.reshape(-1))
    return np.concatenate(parts).reshape(-1, 1).astype(np.float32)


_CACHED_NC = None

FULL = dict(G=16, L=200, D=100, RPP=98, CPI=25)


def _get_nc():
    global _CACHED_NC
    if _CACHED_NC is None:
        _CACHED_NC = build_program(**FULL)
    return _CACHED_NC


def run(word_idxs, embed_table, weights, trace=False, **spmd_kwargs):
    """Build (cached), run on the 8 cores, return (full_out, BassKernelResults)."""
    nc = _get_nc()
    in_maps = make_in_maps(
        word_idxs,
        embed_table,
        weights,
        FULL["G"],
        FULL["L"],
        FULL["D"],
        FULL["RPP"],
        FULL["CPI"],
    )
    res = run_bass_kernel_spmd(
        nc, in_maps, core_ids=list(range(NCORES)), trace=trace, **spmd_kwargs
    )
    out = unshard_out(res.results, FULL["G"])
    return out, res


def kernel(word_idxs, embed_table, weights):
    out, _ = run(word_idxs, embed_table, weights, trace=False)
    return out

